# revision 11
# baseline (speedup 1.0000x reference)
"""Distributed Trainium2 kernel for the attention GEMV chain:

    score = context_vector @ query            [L]         (L=8192, Q=4096)
    attn  = softmax(score)
    s_t   = attn @ context_vector             [Q]
    out   = K_w @ concat(query, s_t)          [Q]

Sharding over 8 NeuronCores:
  - context_vector rows: 1024 per core (score GEMV + partial weighted sums)
  - K_w rows: 512 per core, so each core finishes its own slice of the
    output and no output collective is needed.
  - one AllGather moves 3 flash-softmax group rows per core
    [z_grp(4096) bf16 | m, S fp32 bit-cast | S bf16]; after the gather
    each core finishes the global softmax with a rank-24 exp-weighted
    bf16 matmul that broadcasts s_t to all 128 partitions in PSUM (an
    extra matmul column against the bf16 S values broadcasts sum-exp
    for free); 1/sum(exp) is applied once in the final add.

Schedule rationale (trace-measured on this fleet):
  - The AllGather completes ~37-50us after issue (start skew + a
    fixed-cadence CC protocol, payload-size independent), so phase 1
    exists to issue it ASAP; everything else hides in its shadow.
  - All inputs host-cast to bf16 (rel_err ~2e-3 vs 2e-2 gate).
  - ALL cv/query DMA trigger instructions are emitted before any
    dependent compute: a trigger that waits on a semaphore stalls its
    engine's whole instruction stream, so a z-row staging DMA emitted
    mid-loop would hold back the next cv tile's trigger (measured
    +35us on v3a). With the triggers hoisted, the z/stat DMAs still
    sit between cv and K_w in each queue's FIFO, which is exactly the
    bandwidth priority we want.
  - cv tiles are all kept resident (8 bufs) to avoid pool-recycle
    stalls feeding back into the DMA queues.
  - Row-dots run on the DVE via STT accum (4.42us per [128,4096] tile,
    no 16-bit fast path on this op, measured); the dot chain is the
    phase-1 critical path. Flash groups (4,3,1): the last group needs
    only the final tile, and its PSUM row is evicted in pieces behind
    the per-bank matmuls. Group 1's row is evicted on the DVE instead
    of ACT so it can't collide with the last group's exp/pieces.
  - One PSUM tile serves both the phase-1 z rows and the phase-3 s_t
    broadcast (a mid-kernel PSUM pool swap costs a ~3us engine drain).
  - Tail: stats, S*-column + 8 bank matmuls, then the s_t-half dots
    read the PSUM-resident s_t directly (PSUM in1 has no DVE penalty).
"""
import sys

if "/opt/trn_rl_repo" not in sys.path:
    sys.path.insert(0, "/opt/trn_rl_repo")

from contextlib import ExitStack

import numpy as np
from ml_dtypes import bfloat16

import concourse.bass as bass
import concourse.bacc as bacc
import concourse.mybir as mybir
import concourse.tile as tile
from concourse.bass_isa import ReduceOp
from concourse.bass_utils import run_bass_kernel_spmd

N_CORES = 8
Q = 4096
L = 8192
L_SHARD = L // N_CORES          # 1024 rows of context_vector per core
R_SHARD = Q // N_CORES          # 512 rows of K_w per core
LT = L_SHARD // 128             # 8 cv l-tiles per core
RT = R_SHARD // 128             # 4 K_w r-tiles per core
NB = Q // 512                   # 8 psum banks of 512 fp32
CCW = Q + 8                     # row: z(4096), m(2), S(2), Sbf(1), pad(3)
GROUPS = [(0, 4), (4, 7), (7, 8)]   # flash groups (PSUM rows 0/32/64)
NG = len(GROUPS)
GROWS = N_CORES * NG            # 24 gathered rows
DT = mybir.dt.float32
BF = mybir.dt.bfloat16
MUL = mybir.AluOpType.mult

_NC_CACHE = {}


def build_nc():
    nc = bacc.Bacc("TRN2", target_bir_lowering=False, debug=False,
                   num_devices=N_CORES)

    q_ext = nc.dram_tensor("query", [128, Q], BF, kind="ExternalInput")
    cv_ext = nc.dram_tensor("cv", [L_SHARD, Q], BF, kind="ExternalInput")
    kw_ext = nc.dram_tensor("kw", [R_SHARD, 2 * Q], BF, kind="ExternalInput")
    out_ext = nc.dram_tensor("out", [128, RT], DT, kind="ExternalOutput")

    cc_in = nc.dram_tensor("cc_in", [1, NG * CCW], BF)
    cc_outA = nc.dram_tensor("cc_outA", [N_CORES, NG * CCW], BF,
                             addr_space="Shared")

    with tile.TileContext(nc) as tc, ExitStack() as ctx:
        persist = ctx.enter_context(tc.tile_pool(name="persist", bufs=1))
        smalls = ctx.enter_context(tc.tile_pool(name="smalls", bufs=1))
        late = ctx.enter_context(tc.tile_pool(name="late", bufs=1))
        bigp = ctx.enter_context(tc.tile_pool(name="bigp", bufs=LT))
        prodf = ctx.enter_context(tc.tile_pool(name="prodf", bufs=3))
        psp = ctx.enter_context(tc.tile_pool(name="psp", bufs=1, space="PSUM"))
        ENG = [nc.sync, nc.scalar]

        # ---- all load triggers first: nothing here waits on compute ----
        queryB = persist.tile([128, Q], BF)
        nc.sync.dma_start(out=queryB[:, 0:Q // 2], in_=q_ext[:, 0:Q // 2])
        nc.scalar.dma_start(out=queryB[:, Q // 2:Q], in_=q_ext[:, Q // 2:Q])
        cv_tiles = []
        for t in range(LT):
            cv_t = bigp.tile([128, Q], BF, tag="big", name=f"cv{t}")
            ENG[t % 2].dma_start(out=cv_t,
                                 in_=cv_ext[t * 128:(t + 1) * 128, :])
            cv_tiles.append(cv_t)

        scores = smalls.tile([128, LT], DT)
        nstack = smalls.tile([128, NG], DT)      # negated group maxes
        estack = smalls.tile([128, LT], BF)      # per-tile bf16 exp weights
        sums = smalls.tile([128, NG], DT)        # per-group local expsum
        stats = smalls.tile([128, 2 * NG], DT)   # [m_g, S_g] fp32 pairs
        sbf16 = smalls.tile([128, NG], BF)       # S_g as bf16 (for S* matmul)
        tmp_max = smalls.tile([128, LT], DT)     # per-column partition maxes
        stage = persist.tile([128, Q], BF)       # evicted z rows {0,32,64}
        ones_rep = smalls.tile([GROWS, 128], BF)
        nc.vector.memset(ones_rep, 1.0)
        psum_st = psp.tile([128, Q], DT)         # z rows, later s_t broadcast

        def dot(in0, in1, acc):
            """Row-dot of [128,Q] operands on the DVE via STT accum."""
            pr = prodf.tile([128, Q], BF)
            nc.vector.scalar_tensor_tensor(
                out=pr, in0=in0, scalar=1.0, in1=in1,
                op0=MUL, op1=MUL, accum_out=acc)

        # ---- phase 1: per-group scores, stats, weighted rows ----
        for g, (g0, g1) in enumerate(GROUPS):
            r = 32 * g
            last = g1 - 1
            for t in range(g0, g1):
                dot(cv_tiles[t], queryB, scores[:, t:t + 1])
            nc.gpsimd.partition_all_reduce(
                tmp_max[:, g0:g1], scores[:, g0:g1], 128, ReduceOp.max)
            if g1 - g0 > 1:
                nc.vector.tensor_reduce(
                    out=stats[:, 2 * g:2 * g + 1], in_=tmp_max[:, g0:g1],
                    axis=mybir.AxisListType.X, op=mybir.AluOpType.max)
            else:
                nc.vector.tensor_scalar_mul(
                    stats[:, 2 * g:2 * g + 1], tmp_max[:, g0:g0 + 1], 1.0)
            nc.vector.tensor_scalar_mul(
                nstack[:, g:g + 1], stats[:, 2 * g:2 * g + 1], -1.0)
            for t in range(g0, g1):
                nc.scalar.activation(
                    out=estack[:, t:t + 1], in_=scores[:, t:t + 1],
                    func=mybir.ActivationFunctionType.Exp,
                    bias=nstack[:, g:g + 1], scale=1.0)
            pieces = 4 if g == NG - 1 else 1
            pw = Q // pieces
            for t in range(g0, g1):
                for n in range(NB):
                    sl = slice(n * 512, (n + 1) * 512)
                    nc.tensor.matmul(
                        psum_st[r:r + 1, sl],
                        lhsT=estack[:, t:t + 1],
                        rhs=cv_tiles[t][:, sl],
                        start=(t == g0), stop=(t == last),
                        skip_group_check=True,
                    )
                    # pipeline the last group's eviction behind its
                    # per-bank matmuls (2 banks per ACT piece)
                    if t == last and pieces > 1 and n % 2 == 1:
                        pc = n // 2
                        psl = slice(pc * pw, (pc + 1) * pw)
                        nc.scalar.copy(stage[r:r + 1, psl],
                                       psum_st[r:r + 1, psl])
            if pieces == 1:
                if g == 1:
                    # DVE eviction: keeps ACT free for the last group's
                    # exp + pieces (ACT is the g0 evictor + exp engine)
                    nc.vector.tensor_scalar_mul(
                        stage[r:r + 1, :], psum_st[r:r + 1, :], 1.0)
                else:
                    nc.scalar.copy(stage[r:r + 1, :], psum_st[r:r + 1, :])
            nc.vector.tensor_reduce(
                out=sums[:, g:g + 1], in_=estack[:, g0:g1],
                axis=mybir.AxisListType.X, op=mybir.AluOpType.add)
            nc.gpsimd.partition_all_reduce(
                stats[:, 2 * g + 1:2 * g + 2], sums[:, g:g + 1], 128,
                ReduceOp.add)
            nc.vector.tensor_scalar_mul(
                sbf16[:, g:g + 1], stats[:, 2 * g + 1:2 * g + 2], 1.0)
            # staging DMAs: FIFO position on the sync queue is after all
            # cv loads and before any K_w load
            row_out = bass.AP(tensor=cc_in.ap().tensor, offset=g * CCW,
                              ap=[[0, 1], [1, Q]])
            nc.sync.dma_start(out=row_out, in_=stage[r:r + 1, :])
            nc.sync.dma_start(
                out=cc_in[0:1, g * CCW + Q:g * CCW + Q + 4],
                in_=stats[0:1, 2 * g:2 * g + 2].bitcast(BF))
            nc.sync.dma_start(
                out=cc_in[0:1, g * CCW + Q + 4:g * CCW + Q + 5],
                in_=sbf16[0:1, g:g + 1])

        # ---- phase 2: collective + K_w streams + query-half dots ----
        nc.gpsimd.collective_compute(
            "AllGather",
            mybir.AluOpType.bypass,
            replica_groups=[list(range(N_CORES))],
            ins=[cc_in.ap().opt()],
            outs=[cc_outA.ap().opt()],
        )

        accq = smalls.tile([128, RT], DT)
        accs = smalls.tile([128, RT], DT)
        acc = smalls.tile([128, RT], DT)
        kws_tiles = []
        for p in range(RT):
            kwq_t = bigp.tile([128, Q], BF, tag="big", name=f"kwq{p}")
            ENG[p % 2].dma_start(out=kwq_t,
                                 in_=kw_ext[p * 128:(p + 1) * 128, 0:Q])
            dot(kwq_t, queryB, accq[:, p:p + 1])
            kws_t = bigp.tile([128, Q], BF, tag="big", name=f"kws{p}")
            ENG[(p + 1) % 2].dma_start(out=kws_t,
                                       in_=kw_ext[p * 128:(p + 1) * 128, Q:2 * Q])
            kws_tiles.append(kws_t)

        # ---- phase 3: global softmax combine, s_t broadcast into PSUM ----
        gathered = late.tile([GROWS, CCW], BF)
        half = GROWS // 2
        ginA0 = bass.AP(tensor=cc_outA.ap().tensor, offset=0,
                        ap=[[CCW, half], [1, CCW]])
        ginA1 = bass.AP(tensor=cc_outA.ap().tensor, offset=half * CCW,
                        ap=[[CCW, GROWS - half], [1, CCW]])
        nc.sync.dma_start(out=gathered[0:half, :], in_=ginA0)
        nc.scalar.dma_start(out=gathered[half:GROWS, :], in_=ginA1)

        mg = gathered[:, Q:Q + 2].bitcast(DT)
        sgbf = gathered[0:GROWS, Q + 4:Q + 5]
        mmax = smalls.tile([GROWS, 1], DT)
        nc.gpsimd.partition_all_reduce(mmax, mg, GROWS, ReduceOp.max)
        negM = smalls.tile([GROWS, 1], DT)
        nc.vector.tensor_scalar_mul(negM, mmax, -1.0)
        expm = smalls.tile([GROWS, 1], DT)
        nc.scalar.activation(out=expm, in_=mg,
                             func=mybir.ActivationFunctionType.Exp,
                             bias=negM, scale=1.0)
        alpha_rep = smalls.tile([GROWS, 128], BF)
        nc.vector.tensor_scalar_mul(alpha_rep, ones_rep, expm)
        rS128 = smalls.tile([128, 1], DT)

        # S* broadcast column first (its reciprocal is read before the
        # bank-0 matmul overwrites the column)
        nc.tensor.matmul(psum_st[:, 0:1], lhsT=alpha_rep, rhs=sgbf,
                         start=True, stop=True)
        nc.vector.reciprocal(rS128, psum_st[:, 0:1])
        for n in range(NB):
            sl = slice(n * 512, (n + 1) * 512)
            nc.tensor.matmul(
                psum_st[:, sl],
                lhsT=alpha_rep,
                rhs=gathered[0:GROWS, sl],
                start=True, stop=True,
            )

        # ---- phase 4: K_w s_t-half dots straight from PSUM ----
        for j in range(RT):
            dot(kws_tiles[j], psum_st, accs[:, j:j + 1])

        nc.vector.scalar_tensor_tensor(
            out=acc, in0=accs, scalar=rS128[:, 0:1], in1=accq,
            op0=MUL, op1=mybir.AluOpType.add)
        nc.sync.dma_start(out=out_ext.ap(), in_=acc)

    nc.compile()
    return nc


def get_nc():
    if "nc" not in _NC_CACHE:
        _NC_CACHE["nc"] = build_nc()
    return _NC_CACHE["nc"]


def _shard_inputs(query, context_vector, K_w):
    q1 = np.asarray(query, dtype=np.float32).reshape(1, Q)
    qb = np.ascontiguousarray(
        np.broadcast_to(q1, (128, Q))).astype(bfloat16)
    cvb = np.asarray(context_vector, dtype=np.float32).astype(bfloat16)
    kwb = np.asarray(K_w, dtype=np.float32).astype(bfloat16)
    in_maps = []
    for c in range(N_CORES):
        in_maps.append({
            "query": qb,
            "cv": np.ascontiguousarray(cvb[c * L_SHARD:(c + 1) * L_SHARD]),
            "kw": np.ascontiguousarray(kwb[c * R_SHARD:(c + 1) * R_SHARD]),
        })
    return in_maps


def kernel(query, context_vector, K_w, _trace=False, _trace_kwargs=None):
    nc = get_nc()
    in_maps = _shard_inputs(query, context_vector, K_w)
    res = run_bass_kernel_spmd(nc, in_maps, core_ids=list(range(N_CORES)),
                               trace=_trace, **(_trace_kwargs or {}))
    out = np.concatenate(
        [np.asarray(res.results[c]["out"]).T.reshape(-1) for c in range(N_CORES)]
    ).astype(np.float32)
    if _trace:
        kernel.last_results = res
    return out


# revision 13
# speedup vs baseline: 1.1884x; 1.1884x over previous
"""Distributed Trainium2 kernel for the attention GEMV chain:

    score = context_vector @ query            [L]         (L=8192, Q=4096)
    attn  = softmax(score)
    s_t   = attn @ context_vector             [Q]
    out   = K_w @ concat(query, s_t)          [Q]

Sharding over 8 NeuronCores:
  - context_vector rows: 1024 per core (score GEMV + partial weighted sums)
  - K_w rows: 512 per core, so each core finishes its own slice of the
    output and no output collective is needed.
  - one AllGather moves 3 flash-softmax group rows per core
    [z_grp(4096) bf16 | m, S fp32 bit-cast | S bf16]; after the gather
    each core finishes the global softmax with a rank-24 exp-weighted
    bf16 matmul that broadcasts s_t to all 128 partitions in PSUM (an
    extra matmul column against the bf16 S values broadcasts sum-exp
    for free); 1/sum(exp) is applied once in the final add.

Schedule rationale (trace-measured on this fleet):
  - The AllGather completes ~37-50us after issue (start skew + a
    fixed-cadence CC protocol, payload-size independent), so phase 1
    exists to issue it ASAP; everything else hides in its shadow.
  - All inputs host-cast to bf16 (rel_err ~2e-3 vs 2e-2 gate).
  - ALL cv/query DMA trigger instructions are emitted before any
    dependent compute: a trigger that waits on a semaphore stalls its
    engine's whole instruction stream, so a z-row staging DMA emitted
    mid-loop would hold back the next cv tile's trigger (measured
    +35us on v3a). With the triggers hoisted, the z/stat DMAs still
    sit between cv and K_w in each queue's FIFO, which is exactly the
    bandwidth priority we want.
  - cv tiles are all kept resident (8 bufs) to avoid pool-recycle
    stalls feeding back into the DMA queues.
  - Row-dots run on the DVE via STT accum (4.42us per [128,4096] tile,
    no 16-bit fast path on this op, measured); the dot chain is the
    phase-1 critical path. Flash groups (4,3,1): the last group needs
    only the final tile, and its PSUM row is evicted in pieces behind
    the per-bank matmuls. Group 1's row is evicted on the DVE instead
    of ACT so it can't collide with the last group's exp/pieces.
  - One PSUM tile serves both the phase-1 z rows and the phase-3 s_t
    broadcast (a mid-kernel PSUM pool swap costs a ~3us engine drain).
  - Tail: stats, S*-column + 8 bank matmuls, then the s_t-half dots
    read the PSUM-resident s_t directly (PSUM in1 has no DVE penalty).
"""
import sys

if "/opt/trn_rl_repo" not in sys.path:
    sys.path.insert(0, "/opt/trn_rl_repo")

from contextlib import ExitStack

import numpy as np
from ml_dtypes import bfloat16

import concourse.bass as bass
import concourse.bacc as bacc
import concourse.mybir as mybir
import concourse.tile as tile
from concourse.bass_isa import ReduceOp
from concourse.bass_utils import run_bass_kernel_spmd

N_CORES = 8
Q = 4096
L = 8192
L_SHARD = L // N_CORES          # 1024 rows of context_vector per core
R_SHARD = Q // N_CORES          # 512 rows of K_w per core
LT = L_SHARD // 128             # 8 cv l-tiles per core
RT = R_SHARD // 128             # 4 K_w r-tiles per core
NB = Q // 512                   # 8 psum banks of 512 fp32
CCW = Q + 8                     # row: z(4096), m(2), S(2), Sbf(1), pad(3)
GROUPS = [(0, 4), (4, 7), (7, 8)]   # flash groups (PSUM rows 0/32/64)
NG = len(GROUPS)
GROWS = N_CORES * NG            # 24 gathered rows
DT = mybir.dt.float32
BF = mybir.dt.bfloat16
MUL = mybir.AluOpType.mult

_NC_CACHE = {}


def build_nc():
    nc = bacc.Bacc("TRN2", target_bir_lowering=False, debug=False,
                   num_devices=N_CORES)

    q_ext = nc.dram_tensor("query", [128, Q], BF, kind="ExternalInput")
    cv_ext = nc.dram_tensor("cv", [L_SHARD, Q], BF, kind="ExternalInput")
    kw_ext = nc.dram_tensor("kw", [R_SHARD, 2 * Q], BF, kind="ExternalInput")
    out_ext = nc.dram_tensor("out", [128, RT], DT, kind="ExternalOutput")

    cc_in = nc.dram_tensor("cc_in", [1, NG * CCW], BF)
    cc_outA = nc.dram_tensor("cc_outA", [N_CORES, NG * CCW], BF,
                             addr_space="Shared")

    with tile.TileContext(nc) as tc, ExitStack() as ctx:
        persist = ctx.enter_context(tc.tile_pool(name="persist", bufs=1))
        smalls = ctx.enter_context(tc.tile_pool(name="smalls", bufs=1))
        late = ctx.enter_context(tc.tile_pool(name="late", bufs=1))
        bigp = ctx.enter_context(tc.tile_pool(name="bigp", bufs=LT))
        prodf = ctx.enter_context(tc.tile_pool(name="prodf", bufs=3))
        psp = ctx.enter_context(tc.tile_pool(name="psp", bufs=1, space="PSUM"))
        ENG = [nc.sync, nc.scalar]

        # ---- all load triggers first: nothing here waits on compute ----
        queryB = persist.tile([128, Q], BF)
        nc.sync.dma_start(out=queryB[:, 0:Q // 2], in_=q_ext[:, 0:Q // 2])
        nc.scalar.dma_start(out=queryB[:, Q // 2:Q], in_=q_ext[:, Q // 2:Q])
        cv_tiles = []
        for t in range(LT):
            cv_t = bigp.tile([128, Q], BF, tag="big", name=f"cv{t}")
            ENG[t % 2].dma_start(out=cv_t,
                                 in_=cv_ext[t * 128:(t + 1) * 128, :])
            cv_tiles.append(cv_t)

        scores = smalls.tile([128, LT], DT)
        nstack = smalls.tile([128, NG], DT)      # negated group maxes
        estack = smalls.tile([128, LT], BF)      # per-tile bf16 exp weights
        sums = smalls.tile([128, NG], DT)        # per-group local expsum
        stats = smalls.tile([128, 2 * NG], DT)   # [m_g, S_g] fp32 pairs
        sbf16 = smalls.tile([128, NG], BF)       # S_g as bf16 (for S* matmul)
        tmp_max = smalls.tile([128, LT], DT)     # per-column partition maxes
        stage = persist.tile([128, Q], BF)       # evicted z rows {0,32,64}
        ones_rep = smalls.tile([GROWS, 128], BF)
        nc.vector.memset(ones_rep, 1.0)
        psum_st = psp.tile([128, Q], DT)         # z rows, later s_t broadcast

        def dot(in0, in1, acc):
            """Row-dot of [128,Q] operands on the DVE via STT accum."""
            pr = prodf.tile([128, Q], BF, tag="pr", name="pr")
            nc.vector.scalar_tensor_tensor(
                out=pr, in0=in0, scalar=1.0, in1=in1,
                op0=MUL, op1=MUL, accum_out=acc)

        def dot_ta(in0, in1, acc):
            """Row-dot split: DVE 2x-mode multiply + ACT identity-accum
            reduce (3.7us) - offloads half the dot cost off the DVE."""
            pr = prodf.tile([128, Q], BF, tag="pr", name="prt")
            nc.vector.tensor_mul(pr, in0, in1)
            pr2 = prodf.tile([128, Q], BF, tag="pr", name="prt2")
            nc.scalar.activation(
                out=pr2, in_=pr,
                func=mybir.ActivationFunctionType.Identity,
                bias=0.0, scale=1.0, accum_out=acc)

        # ---- phase 1: per-group scores, stats, weighted rows ----
        for g, (g0, g1) in enumerate(GROUPS):
            r = 32 * g
            last = g1 - 1
            for t in range(g0, g1):
                if t < 3:
                    dot_ta(cv_tiles[t], queryB, scores[:, t:t + 1])
                else:
                    dot(cv_tiles[t], queryB, scores[:, t:t + 1])
            nc.gpsimd.partition_all_reduce(
                tmp_max[:, g0:g1], scores[:, g0:g1], 128, ReduceOp.max)
            if g1 - g0 > 1:
                nc.vector.tensor_reduce(
                    out=stats[:, 2 * g:2 * g + 1], in_=tmp_max[:, g0:g1],
                    axis=mybir.AxisListType.X, op=mybir.AluOpType.max)
            else:
                nc.vector.tensor_scalar_mul(
                    stats[:, 2 * g:2 * g + 1], tmp_max[:, g0:g0 + 1], 1.0)
            nc.vector.tensor_scalar_mul(
                nstack[:, g:g + 1], stats[:, 2 * g:2 * g + 1], -1.0)
            for t in range(g0, g1):
                nc.scalar.activation(
                    out=estack[:, t:t + 1], in_=scores[:, t:t + 1],
                    func=mybir.ActivationFunctionType.Exp,
                    bias=nstack[:, g:g + 1], scale=1.0)
            pieces = 4 if g == NG - 1 else 1
            pw = Q // pieces
            for t in range(g0, g1):
                for n in range(NB):
                    sl = slice(n * 512, (n + 1) * 512)
                    nc.tensor.matmul(
                        psum_st[r:r + 1, sl],
                        lhsT=estack[:, t:t + 1],
                        rhs=cv_tiles[t][:, sl],
                        start=(t == g0), stop=(t == last),
                        skip_group_check=True,
                    )
                    # pipeline the last group's eviction behind its
                    # per-bank matmuls (2 banks per ACT piece)
                    if t == last and pieces > 1 and n % 2 == 1:
                        pc = n // 2
                        psl = slice(pc * pw, (pc + 1) * pw)
                        nc.scalar.copy(stage[r:r + 1, psl],
                                       psum_st[r:r + 1, psl])
            if pieces == 1:
                if g == 1:
                    # DVE eviction: keeps ACT free for the last group's
                    # exp + pieces (ACT is the g0 evictor + exp engine)
                    nc.vector.tensor_scalar_mul(
                        stage[r:r + 1, :], psum_st[r:r + 1, :], 1.0)
                else:
                    nc.scalar.copy(stage[r:r + 1, :], psum_st[r:r + 1, :])
            nc.vector.tensor_reduce(
                out=sums[:, g:g + 1], in_=estack[:, g0:g1],
                axis=mybir.AxisListType.X, op=mybir.AluOpType.add)
            nc.gpsimd.partition_all_reduce(
                stats[:, 2 * g + 1:2 * g + 2], sums[:, g:g + 1], 128,
                ReduceOp.add)
            nc.vector.tensor_scalar_mul(
                sbf16[:, g:g + 1], stats[:, 2 * g + 1:2 * g + 2], 1.0)
            # staging DMAs: FIFO position on the sync queue is after all
            # cv loads and before any K_w load
            row_out = bass.AP(tensor=cc_in.ap().tensor, offset=g * CCW,
                              ap=[[0, 1], [1, Q]])
            nc.sync.dma_start(out=row_out, in_=stage[r:r + 1, :])
            nc.sync.dma_start(
                out=cc_in[0:1, g * CCW + Q:g * CCW + Q + 4],
                in_=stats[0:1, 2 * g:2 * g + 2].bitcast(BF))
            nc.sync.dma_start(
                out=cc_in[0:1, g * CCW + Q + 4:g * CCW + Q + 5],
                in_=sbf16[0:1, g:g + 1])

        # ---- phase 2: collective + K_w streams + query-half dots ----
        nc.gpsimd.collective_compute(
            "AllGather",
            mybir.AluOpType.bypass,
            replica_groups=[list(range(N_CORES))],
            ins=[cc_in.ap().opt()],
            outs=[cc_outA.ap().opt()],
        )

        accq = smalls.tile([128, RT], DT)
        accs = smalls.tile([128, RT], DT)
        acc = smalls.tile([128, RT], DT)
        kws_tiles = []
        for p in range(RT):
            kwq_t = bigp.tile([128, Q], BF, tag="big", name=f"kwq{p}")
            ENG[p % 2].dma_start(out=kwq_t,
                                 in_=kw_ext[p * 128:(p + 1) * 128, 0:Q])
            dot_ta(kwq_t, queryB, accq[:, p:p + 1])
            kws_t = bigp.tile([128, Q], BF, tag="big", name=f"kws{p}")
            ENG[(p + 1) % 2].dma_start(out=kws_t,
                                       in_=kw_ext[p * 128:(p + 1) * 128, Q:2 * Q])
            kws_tiles.append(kws_t)

        # ---- phase 3: global softmax combine, s_t broadcast into PSUM ----
        gathered = late.tile([GROWS, CCW], BF)
        half = GROWS // 2
        ginA0 = bass.AP(tensor=cc_outA.ap().tensor, offset=0,
                        ap=[[CCW, half], [1, CCW]])
        ginA1 = bass.AP(tensor=cc_outA.ap().tensor, offset=half * CCW,
                        ap=[[CCW, GROWS - half], [1, CCW]])
        nc.sync.dma_start(out=gathered[0:half, :], in_=ginA0)
        nc.scalar.dma_start(out=gathered[half:GROWS, :], in_=ginA1)

        mg = gathered[:, Q:Q + 2].bitcast(DT)
        sgbf = gathered[0:GROWS, Q + 4:Q + 5]
        mmax = smalls.tile([GROWS, 1], DT)
        nc.gpsimd.partition_all_reduce(mmax, mg, GROWS, ReduceOp.max)
        negM = smalls.tile([GROWS, 1], DT)
        nc.vector.tensor_scalar_mul(negM, mmax, -1.0)
        expm = smalls.tile([GROWS, 1], DT)
        nc.scalar.activation(out=expm, in_=mg,
                             func=mybir.ActivationFunctionType.Exp,
                             bias=negM, scale=1.0)
        alpha_rep = smalls.tile([GROWS, 128], BF)
        nc.vector.tensor_scalar_mul(alpha_rep, ones_rep, expm)
        rS128 = smalls.tile([128, 1], DT)

        # S* broadcast column first (its reciprocal is read before the
        # bank-0 matmul overwrites the column)
        nc.tensor.matmul(psum_st[:, 0:1], lhsT=alpha_rep, rhs=sgbf,
                         start=True, stop=True)
        nc.vector.reciprocal(rS128, psum_st[:, 0:1])
        for n in range(NB):
            sl = slice(n * 512, (n + 1) * 512)
            nc.tensor.matmul(
                psum_st[:, sl],
                lhsT=alpha_rep,
                rhs=gathered[0:GROWS, sl],
                start=True, stop=True,
            )

        # ---- phase 4: K_w s_t-half dots straight from PSUM; split into
        # column halves so the first dots start after banks 0-3 land ----
        accs_h = smalls.tile([128, 2 * RT], DT)
        H = Q // 2
        for j in range(RT):
            pr = prodf.tile([128, H], BF, tag="prh", name="prh0")
            nc.vector.scalar_tensor_tensor(
                out=pr, in0=kws_tiles[j][:, 0:H], scalar=1.0,
                in1=psum_st[:, 0:H],
                op0=MUL, op1=MUL, accum_out=accs_h[:, 2 * j:2 * j + 1])
        for j in range(RT):
            pr = prodf.tile([128, H], BF, tag="prh", name="prh1")
            nc.vector.scalar_tensor_tensor(
                out=pr, in0=kws_tiles[j][:, H:Q], scalar=1.0,
                in1=psum_st[:, H:Q],
                op0=MUL, op1=MUL, accum_out=accs_h[:, 2 * j + 1:2 * j + 2])
        nc.vector.tensor_add(
            accs, accs_h[:, 0:2 * RT:2], accs_h[:, 1:2 * RT:2])

        nc.vector.scalar_tensor_tensor(
            out=acc, in0=accs, scalar=rS128[:, 0:1], in1=accq,
            op0=MUL, op1=mybir.AluOpType.add)
        nc.sync.dma_start(out=out_ext.ap(), in_=acc)

    nc.compile()
    return nc


def get_nc():
    if "nc" not in _NC_CACHE:
        _NC_CACHE["nc"] = build_nc()
    return _NC_CACHE["nc"]


def _shard_inputs(query, context_vector, K_w):
    q1 = np.asarray(query, dtype=np.float32).reshape(1, Q)
    qb = np.ascontiguousarray(
        np.broadcast_to(q1, (128, Q))).astype(bfloat16)
    cvb = np.asarray(context_vector, dtype=np.float32).astype(bfloat16)
    kwb = np.asarray(K_w, dtype=np.float32).astype(bfloat16)
    in_maps = []
    for c in range(N_CORES):
        in_maps.append({
            "query": qb,
            "cv": np.ascontiguousarray(cvb[c * L_SHARD:(c + 1) * L_SHARD]),
            "kw": np.ascontiguousarray(kwb[c * R_SHARD:(c + 1) * R_SHARD]),
        })
    return in_maps


def kernel(query, context_vector, K_w, _trace=False, _trace_kwargs=None):
    nc = get_nc()
    in_maps = _shard_inputs(query, context_vector, K_w)
    res = run_bass_kernel_spmd(nc, in_maps, core_ids=list(range(N_CORES)),
                               trace=_trace, **(_trace_kwargs or {}))
    out = np.concatenate(
        [np.asarray(res.results[c]["out"]).T.reshape(-1) for c in range(N_CORES)]
    ).astype(np.float32)
    if _trace:
        kernel.last_results = res
    return out
